# revision 75
# baseline (speedup 1.0000x reference)
"""BiMamba Trainium2 kernel (v2 — rebalanced engines).

Sharding: each of the 8 cores owns a 256-channel slice of d_inner for BOTH
directions (fwd+rev share in_proj/out_proj, so the reversed direction's
in_proj output is just a flipped view of the forward one).  Per core:
  - in_proj:  xz[:, slice] = hidden @ W_in[slice].T      (PE, fp16, hT from host)
  - conv+silu: 4 diagonal-matmul taps accumulated in PSUM, then ACT silu
    with per-partition bias (fwd and rev share the padded x buffer; the rev
    conv computes the flipped-layout output via mirrored tap offsets)
  - x_proj:   partial x_dbl summed over cores via per-direction AllReduce
  - dt_proj (PE) + native Softplus (ACT)
  - selective scan per (dir, pb, n): da = exp(A_n*dt)    (ACT, fp16 out)
        dbx = dtx * B_n                                  (DVE fp16 2x)
        h   = tensor_tensor_scan(da, dbx)                (DVE / GpSimd split)
        hC  = h * C_n ; y_ssm = sum_n hC + D*x pair tree (DVE fp16)
  - gate: y = y_ssm * silu(z)  (gz stored once, rev reads reversed)
  - out_proj partial -> ReduceScatter pipelined in 2 dm-chunks
"""

import os
import sys

sys.path.insert(0, "/opt/trn_rl_repo")

import numpy as np
import ml_dtypes

# ---------------------------------------------------------------- constants
P = 128           # partitions
L = 2048          # sequence length
DM = 1024         # d_model
DI = 2048         # d_inner
NST = 16          # d_state
RK = 64           # dt_rank
KCONV = 4         # conv width
NCORES = 8
CH = DI // NCORES          # channels per core per direction = 256
NPT = CH // P              # channel ptiles per core = 2
FB = 512                   # matmul moving free chunk
NFB = L // FB              # 4
PAD = KCONV - 1            # causal pad = 3
NXP = RK + 2 * NST         # 96
HCCH = 1024                # t-chunk for the hC/tree stage
NHC = L // HCCH            # 2
NKB = DM // P              # 8


def build_program(num_cores=NCORES, enable_asserts=False,
                  scan_gp="half", da_f32=False):
    """Build the SPMD Bass program (same NEFF on every core).

    scan_gp: 'none' | 'half' | 'all' — which scans run on GpSimd.
    """
    import concourse.bass as bass
    import concourse.mybir as mybir
    import concourse.tile as tile
    from concourse import bacc
    from contextlib import ExitStack

    dt = mybir.dt
    AF = mybir.ActivationFunctionType
    OP = mybir.AluOpType
    f16 = dt.float16

    nc = bacc.Bacc(
        "TRN2",
        target_bir_lowering=False,
        debug=False,
        enable_asserts=enable_asserts,
        num_devices=num_cores,
    )

    def on_gp(pb, n):
        if scan_gp == "all":
            return True
        if scan_gp == "none":
            return False
        return (n * NPT + pb) % 2 == 0

    # ------------------------------------------------------------- dram I/O
    hT_in = nc.dram_tensor("hT", [DM, L], f16, kind="ExternalInput")
    w_inT = nc.dram_tensor("w_inT", [DM, 2 * CH], f16, kind="ExternalInput")
    w_outT = nc.dram_tensor("w_outT", [CH, DM], f16, kind="ExternalInput")
    ident_in = nc.dram_tensor("ident", [P, P], f16, kind="ExternalInput")
    w_xT = {}
    w_dtT = {}
    conv_diag = {}
    d_diag = {}
    conv_b = {}
    dt_b = {}
    A_in = {}
    for d in ("f", "r"):
        w_xT[d] = nc.dram_tensor(f"w_xT_{d}", [CH, NXP], f16,
                                 kind="ExternalInput")
        w_dtT[d] = nc.dram_tensor(f"w_dtT_{d}", [RK, CH], f16,
                                  kind="ExternalInput")
        conv_diag[d] = nc.dram_tensor(f"conv_diag_{d}", [CH, KCONV * P], f16,
                                      kind="ExternalInput")
        d_diag[d] = nc.dram_tensor(f"d_diag_{d}", [CH, P], f16,
                                   kind="ExternalInput")
        conv_b[d] = nc.dram_tensor(f"conv_b_{d}", [CH, 1], dt.float32,
                                   kind="ExternalInput")
        dt_b[d] = nc.dram_tensor(f"dt_b_{d}", [CH, 1], dt.float32,
                                 kind="ExternalInput")
        A_in[d] = nc.dram_tensor(f"A_{d}", [CH, NST], dt.float32,
                                 kind="ExternalInput")
    out = nc.dram_tensor("out", [L // num_cores, DM], f16,
                         kind="ExternalOutput")

    WPAD = L + 2 * PAD  # padded x width (zeros both ends for the rev conv)
    da_dt = dt.float32 if da_f32 else f16

    with tile.TileContext(nc) as tc:
        ctx = ExitStack()
        with ctx:
            dram = ctx.enter_context(tc.tile_pool(name="dram", bufs=1, space="DRAM"))
            consts = ctx.enter_context(tc.tile_pool(name="consts", bufs=1))
            psum_mm = ctx.enter_context(
                tc.tile_pool(name="psum_mm", bufs=3, space="PSUM"))

            # ---------------------------------------------------- constants
            ident_sb = consts.tile([P, P], f16, name="ident", tag="ident")
            nc.sync.dma_start(ident_sb[:], ident_in[:])
            conv_diag_sb = {}
            d_diag_sb = {}
            conv_b_sb = {}
            dt_b_sb = {}
            A_sb = {}
            for d in ("f", "r"):
                for pb in range(NPT):
                    ps = slice(pb * P, (pb + 1) * P)
                    for nm, store, src, shape, dty in (
                        ("cd", conv_diag_sb, conv_diag, [P, KCONV * P], f16),
                        ("dd", d_diag_sb, d_diag, [P, P], f16),
                        ("cb", conv_b_sb, conv_b, [P, 1], dt.float32),
                        ("db", dt_b_sb, dt_b, [P, 1], dt.float32),
                        ("A", A_sb, A_in, [P, NST], dt.float32),
                    ):
                        t = consts.tile(shape, dty, name=f"{nm}{d}{pb}",
                                        tag=f"{nm}{d}{pb}")
                        nc.sync.dma_start(t[:], src[d][ps, :])
                        store[d, pb] = t
            w_dt_sb = {}
            w_x_sb = {}
            for d in ("f", "r"):
                w_dt_sb[d] = consts.tile([RK, CH], f16, name=f"wdt{d}",
                                         tag=f"wdt{d}")
                nc.sync.dma_start(w_dt_sb[d][:], w_dtT[d][:])
                for pb in range(NPT):
                    t = consts.tile([P, NXP], f16, name=f"wx{d}{pb}",
                                    tag=f"wx{d}{pb}")
                    nc.sync.dma_start(t[:], w_xT[d][pb * P:(pb + 1) * P, :])
                    w_x_sb[d, pb] = t
            w_out_sb = []
            for pb in range(NPT):
                t = consts.tile([P, DM], f16, name=f"wo{pb}", tag=f"wo{pb}")
                nc.sync.dma_start(t[:], w_outT[pb * P:(pb + 1) * P, :])
                w_out_sb.append(t)

            # persistent activation buffers
            gz_pool = ctx.enter_context(tc.tile_pool(name="gzp", bufs=1))
            gz = {}           # gz[pb]: silu(z), shared by both directions
            for pb in range(NPT):
                gz[pb] = gz_pool.tile([P, L], f16, name=f"gz{pb}",
                                      tag=f"gz{pb}")
            oev_pool = ctx.enter_context(tc.tile_pool(name="oevp", bufs=2))

            xdbl_part = {}
            xdbl_sum = {}
            for d in ("f", "r"):
                for th in range(2):
                    xdbl_part[d, th] = dram.tile(
                        [NXP, L // 2], f16, name=f"xdp{d}{th}",
                        tag=f"xdp{d}{th}")
                    xdbl_sum[d, th] = dram.tile(
                        [NXP, L // 2], f16, addr_space="Shared",
                        name=f"xds{d}{th}", tag=f"xds{d}{th}")
            pout = dram.tile([L, DM], f16, name="pout", tag="pout")
            pout_rs = dram.tile([L // num_cores, DM], f16, name="prs",
                                tag="prs")
            dummy_p = dram.tile([1, 512], f16, name="dmyp", tag="dmyp")
            dummy_s = dram.tile([1, 512], f16, addr_space="Shared",
                                name="dmys", tag="dmys")
            # tiny collective issued first: absorbs inter-core start skew so
            # the first real AllReduce doesn't pay for it
            nc.gpsimd.collective_compute(
                "AllReduce", OP.add,
                replica_groups=[list(range(num_cores))],
                ins=[dummy_p[:].opt()], outs=[dummy_s[:].opt()])

            psum_y = ctx.enter_context(
                tc.tile_pool(name="psum_y", bufs=1, space="PSUM"))
            xc_pool = ctx.enter_context(tc.tile_pool(name="xcp", bufs=1))
            dt_pool = ctx.enter_context(tc.tile_pool(name="dtp", bufs=4))
            dtx_pool = ctx.enter_context(tc.tile_pool(name="dtxp", bufs=4))
            xdbl_pool = ctx.enter_context(tc.tile_pool(name="xdblp", bufs=1))
            et_pool = ctx.enter_context(tc.tile_pool(name="etp", bufs=2))
            # stage-limited pools (freed once early phases are emitted)
            ctxA = ExitStack()
            hT_pool = ctxA.enter_context(tc.tile_pool(name="hTp", bufs=1))
            w_in_pool = ctxA.enter_context(tc.tile_pool(name="winp", bufs=1))
            ctxB = ExitStack()
            xpad_pool = ctxB.enter_context(tc.tile_pool(name="xpadp", bufs=1))
            xev_pool = ctxB.enter_context(tc.tile_pool(name="xevp", bufs=2))


            # ------------------------------------------- stage 1: loads
            hT = [hT_pool.tile([P, L], f16, name=f"hT{k}", tag=f"hT{k}")
                  for k in range(NKB)]
            w_in_sb = [w_in_pool.tile([P, 2 * CH], f16, name=f"win{k}",
                                      tag=f"win{k}") for k in range(NKB)]
            # chunk the big loads so they spread across DMA engines instead
            # of riding one engine each at ~23us
            for k in range(NKB):
                nc.sync.dma_start(w_in_sb[k][:], w_inT[k * P:(k + 1) * P, :])
                for q in range(4):
                    cs = slice(q * (L // 4), (q + 1) * (L // 4))
                    nc.sync.dma_start(hT[k][:, cs],
                                      hT_in[k * P:(k + 1) * P, cs])

            xpad = [xpad_pool.tile([P, WPAD], f16, name=f"xpad{pb}",
                                   tag=f"xpad{pb}") for pb in range(NPT)]
            for pb in range(NPT):
                nc.vector.memset(xpad[pb][:, 0:PAD], 0.0)
                nc.vector.memset(xpad[pb][:, PAD + L:WPAD], 0.0)

            # ------------------------------------------- stage 2: in_proj
            # x half first (mb<NPT) so convs can start asap; the z half is
            # deferred until after xproj_f so AR_f goes on the wire early.
            # k inside so the first matmuls only wait for the first hT tiles.
            def in_proj_x():
                for mb in range(NPT):
                    pms = [psum_y.tile([P, FB], dt.float32, name="mm",
                                       tag=f"yac{fb}") for fb in range(NFB)]
                    for k in range(NKB):
                        for fb in range(NFB):
                            nc.tensor.matmul(
                                pms[fb][:],
                                w_in_sb[k][:, mb * P:(mb + 1) * P],
                                hT[k][:, fb * FB:(fb + 1) * FB],
                                start=(k == 0),
                                stop=(k == NKB - 1),
                            )
                    for fb in range(NFB):
                        nc.scalar.copy(
                            xpad[mb][:, PAD + fb * FB: PAD + (fb + 1) * FB],
                            pms[fb][:])

            # z half: matmuls emitted early (mm-tag psums, 2 at a time); the
            # silu evictions are emitted later as hooks inside the first scan
            # halves so they never head-of-line-block the ACT da stream.
            z_psums = {}

            def z_mm(mb, fbs):
                pms = [psum_mm.tile([P, FB], dt.float32, name="mmz",
                                    tag="mm") for _ in fbs]
                for k in range(NKB):
                    for i, fb in enumerate(fbs):
                        nc.tensor.matmul(
                            pms[i][:],
                            w_in_sb[k][:, mb * P:(mb + 1) * P],
                            hT[k][:, fb * FB:(fb + 1) * FB],
                            start=(k == 0),
                            stop=(k == NKB - 1),
                        )
                for i, fb in enumerate(fbs):
                    z_psums[mb, fb] = pms[i]

            def z_ev(mb, fbs):
                def fn():
                    pb = mb - NPT
                    for fb in fbs:
                        nc.scalar.activation(
                            gz[pb][:, fb * FB:(fb + 1) * FB],
                            z_psums[mb, fb][:], AF.Silu)
                return fn

            # ------------------------------------------- stage 3: conv+xproj
            xc = {}

            def conv_block(d, chunks):
                """Depthwise conv via 4 diagonal-matmul taps + silu(bias).

                fwd tap b reads xpad offset b; rev tap b reads offset PAD+b
                (host packs rev diag block b = diag(w_r[:, PAD-b])), and the
                silu writes the flipped-layout output."""
                base = 0 if d == "f" else PAD
                for pb in range(NPT):
                    if (d, pb) not in xc:
                        xc[d, pb] = xc_pool.tile([P, L], f16,
                                                 name=f"xc{d}{pb}",
                                                 tag=f"xc{d}{pb}")
                    t = xc[d, pb]
                    for c in chunks:
                        pm = psum_mm.tile([P, FB], dt.float32, name="mmc",
                                          tag="mm")
                        for b in range(KCONV):
                            off = base + b + c * FB
                            nc.tensor.matmul(
                                pm[:],
                                conv_diag_sb[d, pb][:, b * P:(b + 1) * P],
                                xpad[pb][:, off:off + FB],
                                start=(b == 0),
                                stop=(b == KCONV - 1),
                            )
                        if d == "f":
                            dst = t[:, c * FB:(c + 1) * FB]
                        else:
                            dst = t[:, ::-1][:, c * FB:(c + 1) * FB]
                        nc.scalar.activation(dst, pm[:], AF.Silu,
                                             bias=conv_b_sb[d, pb][:, 0:1])

            def xproj_block(d, th):
                for j in range(2):
                    fb = th * 2 + j
                    pm = psum_mm.tile([NXP, FB], dt.float32, name="mmx",
                                      tag="mm")
                    for pb in range(NPT):
                        nc.tensor.matmul(
                            pm[:],
                            w_x_sb[d, pb][:],
                            xc[d, pb][:, fb * FB:(fb + 1) * FB],
                            start=(pb == 0),
                            stop=(pb == NPT - 1),
                        )
                    xev = xev_pool.tile([NXP, FB], f16, name="xev",
                                        tag="xev")
                    nc.scalar.copy(xev[:], pm[:])
                    nc.sync.dma_start(
                        xdbl_part[d, th][:, j * FB:(j + 1) * FB], xev[:])
                nc.gpsimd.collective_compute(
                    "AllReduce",
                    OP.add,
                    replica_groups=[list(range(num_cores))],
                    ins=[xdbl_part[d, th][:].opt()],
                    outs=[xdbl_sum[d, th][:].opt()],
                )

            dt_sb = {}
            dtx = {}
            y = {}

            def dt_block(d, th):
                """Load summed x_dbl half (f16), dt_proj + softplus, dtx."""
                t0 = th * (L // 2)
                tsl = slice(t0, t0 + L // 2)
                xdbl = xdbl_pool.tile([NXP, L // 2], f16, name="xdbl",
                                      tag=f"xdbl{th}")
                nc.sync.dma_start(xdbl[:], xdbl_sum[d, th][:])
                for pb in range(NPT):
                    if (d, pb) not in dt_sb:
                        dt_sb[d, pb] = dt_pool.tile([P, L], f16, name="dtt",
                                                    tag="dtt")
                        dtx[d, pb] = dtx_pool.tile([P, L], f16, name="dtx",
                                                   tag="dtx")
                    t = dt_sb[d, pb]
                    ets = []
                    for j in range(2):
                        pm = psum_mm.tile([P, FB], dt.float32, name="mm",
                                          tag="mm")
                        nc.tensor.matmul(
                            pm[:],
                            w_dt_sb[d][:, pb * P:(pb + 1) * P],
                            xdbl[0:RK, j * FB:(j + 1) * FB],
                            start=True, stop=True)
                        et = et_pool.tile([P, FB], dt.float32, name="etmp",
                                          tag="etmp")
                        nc.scalar.activation(
                            et[:], pm[:], AF.Exp, bias=dt_b_sb[d, pb][:, 0:1])
                        ets.append(et)
                    for j in range(2):
                        nc.scalar.activation(
                            t[:, t0 + j * FB:t0 + (j + 1) * FB], ets[j][:],
                            AF.Ln, bias=1.0)
                    nc.vector.tensor_mul(dtx[d, pb][:, tsl], t[:, tsl],
                                         xc[d, pb][:, tsl])

            in_proj_x()
            conv_block("f", (0, 1))
            xproj_block("f", 0)
            conv_block("f", (2, 3))
            xproj_block("f", 1)
            # rev conv writes flipped: xc_r cols [0,L/2) come from chunks 2,3
            conv_block("r", (2, 3))
            xproj_block("r", 0)
            conv_block("r", (0, 1))
            xproj_block("r", 1)
            dt_block("f", 0)
            dt_block("f", 1)
            z_mm(NPT, (0, 1))
            z_mm(NPT, (2, 3))
            z_mm(NPT + 1, (0, 1))
            z_mm(NPT + 1, (2, 3))
            ctxB.close()
            ctxA.close()

            # ------------------------------------------- stage 4: scan
            bbc_pool = ctx.enter_context(tc.tile_pool(name="bbcp", bufs=2))
            cbc_pool = ctx.enter_context(tc.tile_pool(name="cbcp", bufs=2))
            da_pool = ctx.enter_context(tc.tile_pool(name="dap", bufs=4))
            dbx_pool = ctx.enter_context(tc.tile_pool(name="dbxp", bufs=2))
            h_pool = ctx.enter_context(tc.tile_pool(name="hp", bufs=3))
            hc_pool = ctx.enter_context(tc.tile_pool(name="hcp", bufs=4))
            y_pool = ctx.enter_context(tc.tile_pool(name="yp", bufs=1))

            TH = L // 2          # time-half length
            NCH = TH // FB       # 512-col psum chunks per half = 2

            hend_pool = ctx.enter_context(tc.tile_pool(name="hendp", bufs=2))
            hend = {}

            def scan_half(d, th, hooks=None):
                """One time-half of the selective scan for direction d.

                n-outer / pb-inner so the B/C broadcasts are shared by both
                channel ptiles; the scan chains across halves via per-lane
                `initial` states; the n-sum runs on the PE (identity matmuls
                accumulating into PSUM, seeded with D*x via a diagonal
                matmul); hC runs on GpSimd."""
                t0 = th * TH
                tsl = slice(t0, t0 + TH)
                if th == 0:
                    for pb in range(NPT):
                        hend[d, pb] = hend_pool.tile(
                            [P, NST], f16, name=f"he{pb}", tag=f"he{pb}")
                accs = {}
                for pb in range(NPT):
                    for c in range(NCH):
                        pm = psum_y.tile([P, FB], dt.float32, name="yac",
                                         tag=f"yac{pb * NCH + c}")
                        nc.tensor.matmul(
                            pm[:], d_diag_sb[d, pb][:],
                            xc[d, pb][:, t0 + c * FB:t0 + (c + 1) * FB],
                            start=True, stop=False, skip_group_check=True)
                        accs[pb, c] = pm
                NG = 4                     # n-states per batched DVE op
                for n0 in range(0, NST, NG):
                    if hooks and n0 in hooks:
                        hooks[n0]()
                    # per-row broadcast DMAs (4 engines in parallel) filling
                    # one batched [P, NG*TH] tile for the wide DVE ops
                    bb = bbc_pool.tile([P, NG * TH], f16, name="bbc",
                                       tag="bbc")
                    cbt = cbc_pool.tile([P, NG * TH], f16, name="cbc",
                                        tag="cbc")
                    for j in range(NG):
                        rb = xdbl_sum[d, th][RK + n0 + j:RK + n0 + j + 1, :]
                        rc = xdbl_sum[d, th][
                            RK + NST + n0 + j:RK + NST + n0 + j + 1, :]
                        nc.sync.dma_start(
                            bb[:, j * TH:(j + 1) * TH],
                            bass.AP(rb.tensor, rb.offset, [[0, P], [1, TH]]))
                        nc.sync.dma_start(
                            cbt[:, j * TH:(j + 1) * TH],
                            bass.AP(rc.tensor, rc.offset, [[0, P], [1, TH]]))
                    for pb in range(NPT):
                        # dbx for NG states in one op: dtx repeated via a
                        # stride-0 middle AP dim
                        dtxs = dtx[d, pb][:, tsl]
                        dtx_rep = bass.AP(
                            dtxs.tensor, dtxs.offset,
                            [list(dtxs.ap[0]), [0, NG], [1, TH]])
                        dbx = dbx_pool.tile([P, NG * TH], f16, name="dbx",
                                            tag="dbx")
                        nc.vector.tensor_tensor(dbx[:], dtx_rep, bb[:],
                                                OP.mult)
                        h4 = h_pool.tile([P, NG * TH], f16, name="h",
                                         tag="h")
                        for j in range(NG):
                            n = n0 + j
                            da = da_pool.tile([P, TH], da_dt, name="da",
                                              tag="da")
                            nc.scalar.activation(
                                da[:], dt_sb[d, pb][:, tsl], AF.Exp,
                                scale=A_sb[d, pb][:, n:n + 1])
                            hj = h4[:, j * TH:(j + 1) * TH]
                            init = (0.0 if th == 0
                                    else hend[d, pb][:, n:n + 1])
                            nc.vector.tensor_tensor_scan(
                                hj, da[:], dbx[:, j * TH:(j + 1) * TH],
                                init, OP.mult, OP.add)
                            if th == 0:
                                nc.vector.tensor_scalar_add(
                                    hend[d, pb][:, n:n + 1],
                                    h4[:, (j + 1) * TH - 1:(j + 1) * TH],
                                    0.0)
                        hc = hc_pool.tile([P, NG * TH], f16, name="hc",
                                          tag="hc")
                        nc.vector.tensor_mul(hc[:], h4[:], cbt[:])
                        for j in range(NG):
                            for c in range(NCH):
                                nc.tensor.matmul(
                                    accs[pb, c][:], ident_sb[:],
                                    hc[:, j * TH + c * FB:
                                        j * TH + (c + 1) * FB],
                                    start=False,
                                    stop=(n0 + j == NST - 1),
                                    skip_group_check=True)
                for pb in range(NPT):
                    if th == 0 and d == "f":
                        yt = y_pool.tile([P, L], f16, name="y", tag=f"y{pb}")
                        y["f", pb] = yt
                    elif th == 0:
                        yt = y_pool.tile([P, L], f16, name="yr", tag=f"yr{pb}")
                        y["r", pb] = yt
                    yt = y[d, pb]
                    gzt = gz[pb] if d == "f" else gz[pb][:, ::-1]
                    for c in range(NCH):
                        sl = slice(t0 + c * FB, t0 + (c + 1) * FB)
                        nc.vector.tensor_mul(yt[:, sl], accs[pb, c][:],
                                             gzt[:, sl])

            def out_proj_fb_tbs(fb, tbs):
                for tb in tbs:
                    pm = psum_mm.tile([P, FB], dt.float32, name="mm",
                                      tag="mm")
                    for pb in range(NPT):
                        nc.tensor.matmul(
                            pm[:],
                            ycb[pb][:, tb * P:(tb + 1) * P],
                            w_out_sb[pb][:, fb * FB:(fb + 1) * FB],
                            start=(pb == 0),
                            stop=(pb == NPT - 1),
                        )
                    oev = oev_pool.tile([P, FB], f16, name="oev",
                                        tag="oev")
                    nc.scalar.copy(oev[:], pm[:])
                    nc.sync.dma_start(
                        pout[tb * P:(tb + 1) * P, fb * FB:(fb + 1) * FB],
                        oev[:])

            def out_proj_tbs(tbs):
                """out_proj partial for the given token blocks (both fb)."""
                for fb in range(DM // FB):
                    out_proj_fb_tbs(fb, tbs)

            scan_half("f", 0, hooks={4: z_ev(NPT, (0, 1)),
                                     8: z_ev(NPT, (2, 3)),
                                     12: z_ev(NPT + 1, (0, 1))})
            dt_block("r", 0)
            scan_half("f", 1, hooks={0: z_ev(NPT + 1, (2, 3))})
            dt_block("r", 1)
            scan_half("r", 0)

            # tokens [L/2, L): y_f second half + y_r first half (flipped);
            # the out_proj for those tokens is emitted mid-way through the
            # last scan half so the PE works on it between accumulations.
            ycb = {}
            for pb in range(NPT):
                t = y["f", pb]
                nc.vector.tensor_add(
                    t[:, TH:L], t[:, TH:L],
                    y["r", pb][:, ::-1][:, TH:L])
                ycb[pb] = t

            scan_half("r", 1,
                      hooks={8: lambda: out_proj_tbs(range(L // P // 2,
                                                           L // P))})
            for pb in range(NPT):
                t = y["f", pb]
                nc.vector.tensor_add(
                    t[:, 0:TH], t[:, 0:TH],
                    y["r", pb][:, ::-1][:, 0:TH])

            # ---------------------------- stage 5: out_proj half1 + RS + out
            out_proj_tbs(range(L // P // 2))
            nc.gpsimd.collective_compute(
                "ReduceScatter",
                OP.add,
                replica_groups=[list(range(num_cores))],
                ins=[pout[:].opt()],
                outs=[pout_rs[:].opt()],
            )
            nc.sync.dma_start(out[:], pout_rs[:])

    return nc


# ---------------------------------------------------------------- host side
def _make_in_maps(inputs):
    """Slice/transpose the full inputs into per-core input dicts."""
    h = np.asarray(inputs["hidden_states"], dtype=np.float32).reshape(L, DM)
    hT = np.ascontiguousarray(h.T).astype(np.float16)
    w_in = np.asarray(inputs["in_proj_w"], dtype=np.float32)     # (2DI, DM)
    w_out = np.asarray(inputs["out_proj_w"], dtype=np.float32)   # (DM, DI)

    ident = np.eye(P, dtype=np.float16)
    in_maps = []
    for c in range(NCORES):
        sl = slice(c * CH, (c + 1) * CH)
        m = {"hT": hT, "ident": ident}
        w_slice = np.concatenate(
            [w_in[sl, :], w_in[DI + c * CH: DI + (c + 1) * CH, :]], axis=0)
        m["w_inT"] = np.ascontiguousarray(w_slice.T).astype(np.float16)
        m["w_outT"] = np.ascontiguousarray(
            w_out[:, sl].T).astype(np.float16)                    # (CH, DM)
        for d, tag in (("f", "_f"), ("r", "_r")):
            w_x = np.asarray(inputs[f"x_proj_w{tag}"], dtype=np.float32)
            m[f"w_xT_{d}"] = np.ascontiguousarray(
                w_x[:, sl].T).astype(np.float16)                  # (CH, 96)
            w_dt = np.asarray(inputs[f"dt_proj_w{tag}"], dtype=np.float32)
            m[f"w_dtT_{d}"] = np.ascontiguousarray(
                w_dt[sl, :].T).astype(np.float16)                 # (RK, CH)
            cw = np.asarray(inputs[f"conv_w{tag}"], dtype=np.float32)[sl, :]
            diag = np.zeros((CH, KCONV * P), dtype=np.float16)
            for pb in range(NPT):
                for b in range(KCONV):
                    k = b if d == "f" else (KCONV - 1 - b)
                    blk = np.diag(cw[pb * P:(pb + 1) * P, k])
                    diag[pb * P:(pb + 1) * P, b * P:(b + 1) * P] = blk
            m[f"conv_diag_{d}"] = diag
            Dv = np.asarray(inputs[f"D{tag}"], dtype=np.float32)[sl]
            ddiag = np.zeros((CH, P), dtype=np.float16)
            for pb in range(NPT):
                ddiag[pb * P:(pb + 1) * P, :] = np.diag(
                    Dv[pb * P:(pb + 1) * P])
            m[f"d_diag_{d}"] = ddiag
            m[f"conv_b_{d}"] = np.ascontiguousarray(
                np.asarray(inputs[f"conv_b{tag}"], dtype=np.float32)[sl, None])
            m[f"dt_b_{d}"] = np.ascontiguousarray(
                np.asarray(inputs[f"dt_proj_b{tag}"], dtype=np.float32)[sl, None])
            m[f"A_{d}"] = np.ascontiguousarray(
                -np.exp(np.asarray(inputs[f"A_log{tag}"], dtype=np.float32)[sl, :]))
        in_maps.append(m)
    return in_maps


_CACHED = {}


def _install_ntff_hook_shim():
    """The agent image's antenv lacks axon_hooks; provide it and register
    the ctypes-based NTFF profile hook from trn_agent_boot."""
    import types
    try:
        import antenv.axon_hooks  # noqa: F401
        return
    except ImportError:
        pass
    import antenv
    mod = types.ModuleType("antenv.axon_hooks")
    _state = {"h": None}
    mod.get_axon_ntff_profile_hook = lambda: _state["h"]
    mod.set_axon_ntff_profile_hook = lambda h: _state.__setitem__("h", h)
    sys.modules["antenv.axon_hooks"] = mod
    antenv.axon_hooks = mod
    try:
        from trn_agent_boot.trn_boot import _ntff_profile_via_ctypes
        hook = _ntff_profile_via_ctypes("/opt/axon/libaxon_pjrt.so")
        if hook is not None:
            mod.set_axon_ntff_profile_hook(hook)
    except Exception:
        pass


def _install_hook_err_capture():
    """Wrap the neuronx_cc hook so compile errors land in hook_err.log
    instead of being swallowed by the PJRT boundary."""
    import traceback
    import concourse.bass2jax as b2j
    if getattr(b2j, "_err_capture_installed", False):
        return
    orig = b2j.neuronx_cc_hook

    def wrapped(*a):
        try:
            return orig(*a)
        except Exception:
            with open("/tmp/hook_err.log", "w") as f:
                f.write(traceback.format_exc())
            raise

    b2j.neuronx_cc_hook = wrapped
    b2j._err_capture_installed = True


def kernel(**inputs):
    from concourse.bass_utils import run_bass_kernel_spmd

    _install_ntff_hook_shim()
    _install_hook_err_capture()

    if "nc" not in _CACHED:
        from concourse.bass_interp import get_hw_module
        nc = build_program(
            scan_gp=os.environ.get("KERNEL_SCAN_GP", "none"),
            da_f32=bool(int(os.environ.get("KERNEL_DA_F32", "0"))))
        nc.finalize()  # bacc: register allocation, library/ACT-table loads
        nc.m = get_hw_module(nc.m)  # strip sim-only callback instructions
        _CACHED["nc"] = nc
    nc = _CACHED["nc"]

    in_maps = _make_in_maps(inputs)
    res = run_bass_kernel_spmd(
        nc, in_maps, core_ids=list(range(NCORES)),
        trace=bool(int(os.environ.get("KERNEL_TRACE", "0"))),
    )
    _CACHED["last_result"] = res
    outs = [res.results[c]["out"] for c in range(NCORES)]
    full = np.concatenate(outs, axis=0).reshape(1, L, DM).astype(np.float32)
    return full


if __name__ == "__main__":
    nc = build_program()
    try:
        n = sum(len(bb.instructions) for bb in nc.main_func.blocks)
    except Exception:
        n = "?"
    print("build ok; instructions:", n)
